# revision 24
# baseline (speedup 1.0000x reference)
"""GRU cell kernel for Trainium2, data-parallel over batch across 8 NeuronCores.

Reference computation (B=8192, D=H=1024), per batch row:
    z = sigmoid(inp@wz + state@uz + bz)
    r = sigmoid(inp@wr + state@ur + br)
    h_ = tanh(inp@wx + bx + (state@wh) * r)
    hid = (1-z)*h_ + state*z

Strategy: each core takes a 1024-row batch shard. The z/r projections fuse
into one [1024,2048]@[2048,2048] GEMM computed in fp8(e4m3) DoubleRow mode:
one PE instruction contracts TWO 128-deep k-tiles in the time a bf16
instruction contracts one. The hh GEMM also runs fp8 DoubleRow (its
quantization noise is attenuated by the r gate before the tanh), and 2 of
the xh GEMM's 8 k-tiles ride fp8 too; the rest of xh stays bf16 (it feeds
tanh at slope ~1 and dominates the error budget). The output is stored
bf16 and upcast to f32 on the host. Measured output rel err 1.863e-2 vs
the 2e-2 budget (deterministic). All PSUM accumulation is fp32; the fp8
scales (x*32, w*1600) divide out inside the sigmoid/tanh input scales —
wx is pre-scaled by SX*SW on the host so phh and pxh share one scale and
the tanh descale is free. Inputs load as large contiguous DMA descriptors
in exact consumption order on the Sync HWDGE ring (descriptor issue costs
~600ns of engine time, and per-ring transfers are FIFO, so this is a
just-in-time feed at full HBM bandwidth); the first fp8 act chunks ride
the parallel Scalar ring so the opening w/x pair lands concurrently.
Out-DMAs reuse the then-idle Sync ring. 12 warmup matmuls on scratch data
ride out the PE HAM throttle (~3.4us activity window at 1.2GHz cold clock)
while the first inputs land. Epilogue engine split: Vector does the
PSUM-reading mul/add chain, Scalar does sigmoid/tanh/(1-z), GpSimd does
z*state. The last tile's hh*r is hoisted to the phase start and its xh
GEMM is split into 384+128-col accumulation groups so only a short
128-col add/tanh/mul/add chain plus one small DMA remains after the
final matmul.
"""

import os
import sys
import types

sys.path.insert(0, "/opt/trn_rl_repo")

import numpy as np

# trace=True under axon needs antenv.axon_hooks, absent from this image.
# Register the same ctypes-backed NTFF hook trn_boot would have installed.
if "antenv.axon_hooks" not in sys.modules:
    _m = types.ModuleType("antenv.axon_hooks")
    _m._hook = None

    def _set_hook(h):
        _m._hook = h

    def _get_hook():
        return _m._hook

    _m.set_axon_ntff_profile_hook = _set_hook
    _m.get_axon_ntff_profile_hook = _get_hook
    sys.modules["antenv.axon_hooks"] = _m
    try:
        from trn_agent_boot.trn_boot import _ntff_profile_via_ctypes

        _m.set_axon_ntff_profile_hook(
            _ntff_profile_via_ctypes("/opt/axon/libaxon_pjrt.so")
        )
    except Exception:
        pass

import concourse.bacc as bacc
import concourse.tile as tile
from concourse import mybir
from concourse.bass_utils import run_bass_kernel_spmd

N_CORES = 8
B, D, H = 8192, 1024, 1024
BL = B // N_CORES  # batch rows per core
P = 128  # partitions
NF = 512  # matmul free dim (one PSUM bank of fp32)
KD = D // P  # k-tiles per 1024 contraction
MT = BL // P  # batch m-tiles per core
F32 = mybir.dt.float32
BF16 = mybir.dt.bfloat16
FP8 = mybir.dt.float8e4
DR = mybir.MatmulPerfMode.DoubleRow
SX, SW = 32.0, 1600.0  # fp8 quantization scales for acts / weights

_CACHE = {}


def _build_fast():
    """No-bias fast path: zr + hh GEMMs in fp8 DoubleRow, xh in bf16."""
    nc = bacc.Bacc("TRN2", target_bir_lowering=False, debug=False)

    # Host-retiled layouts (see kernel()): row index is (group*128 + p),
    # then [ktile, col] so one partition's span is one contiguous run.
    xt = nc.declare_dram_parameter("xt", [2 * P, KD * NF], BF16, isOutput=False)
    x8d = nc.declare_dram_parameter("x8", [2 * P, KD, NF], FP8, isOutput=False)
    s8d = nc.declare_dram_parameter("s8", [2 * P, KD, NF], FP8, isOutput=False)
    wzr8d = nc.declare_dram_parameter("wzr8", [4 * P, 2 * KD, NF], FP8, isOutput=False)
    wh8d = nc.declare_dram_parameter("wh8", [2 * P, KD, NF], FP8, isOutput=False)
    wxt = nc.declare_dram_parameter("wxt", [2 * P, KD * NF], BF16, isOutput=False)
    wx8d = nc.declare_dram_parameter("wx8", [2 * P, 2, NF], FP8, isOutput=False)
    stb = nc.declare_dram_parameter("stb", [P, 2, MT * NF], BF16, isOutput=False)
    out = nc.declare_dram_parameter("out", [BL, H], BF16, isOutput=True)

    with tile.TileContext(nc) as tc:
        with (
            tc.tile_pool(name="acts", bufs=1) as acts,
            tc.tile_pool(name="wts", bufs=1) as wts,
            tc.tile_pool(name="stash", bufs=1) as stash,
            tc.tile_pool(name="stp", bufs=8) as stp,
            tc.tile_pool(name="tmp", bufs=3) as tmp,
            tc.tile_pool(name="small", bufs=1) as small,
            tc.tile_pool(name="ps", bufs=8, space="PSUM") as ps,
        ):
            # Warmup matmuls on scratch data ride out the PE p-state ramp and
            # HAM throttle while the first input DMAs land (~10us ETA). Sized
            # to end right as the first chunks arrive so PE stays busy
            # through the HAM activity window without delaying real work.
            warm_sb = small.tile([P, 2 * P], BF16, tag="warm_sb")
            nc.vector.memset(warm_sb, 0.0)
            warm_ps = ps.tile([P, 2 * P], F32, tag="ps", name="warm_ps")
            for i in range(20):
                nc.tensor.matmul(
                    warm_ps, warm_sb[:, :P], warm_sb, start=True, stop=True
                )

            # Resident tiles. bf16 x-acts serve the xh GEMM; fp8 copies
            # (3D: [p, ktile, col]) serve the DoubleRow zr + hh GEMMs.
            xact = [acts.tile([P, KD * NF], BF16, tag=f"xa{h}", name=f"xa{h}") for h in range(2)]
            x8a = [acts.tile([P, KD, NF], FP8, tag=f"x8{h}", name=f"x8{h}") for h in range(2)]
            s8a = [acts.tile([P, KD, NF], FP8, tag=f"s8{h}", name=f"s8{h}") for h in range(2)]
            wzr8_sb = [
                wts.tile([P, 2 * KD, NF], FP8, tag=f"wz8{g}", name=f"wz8{g}")
                for g in range(4)
            ]
            wh8_sb = [wts.tile([P, KD, NF], FP8, tag=f"wh8{c}", name=f"wh8{c}") for c in range(2)]
            wx_sb = [wts.tile([P, KD * NF], BF16, tag=f"wx{c}", name=f"wx{c}") for c in range(2)]
            wx8_sb = [wts.tile([P, 2, NF], FP8, tag=f"wq{c}", name=f"wq{c}") for c in range(2)]
            st_sb = [acts.tile([P, MT * NF], BF16, tag=f"st{c}", name=f"st{c}") for c in range(2)]

            # DMA issue plan: everything rides the Sync HWDGE ring in exact
            # zr-consumption order (per-ring transfers are FIFO), so the
            # early feed gets the full HBM bandwidth just-in-time with no
            # second-ring contention. 256KB first chunks let the first zr
            # matmul start ~3us earlier than 1MB ones would. Scalar stays
            # free for sigmoids/tanh; out-DMAs ride Sync later (it is idle
            # after these 19 issues) plus Scalar for the final drain.
            # The x-side fp8 acts for the first zr half ride the Scalar
            # ring so the very first w/x chunk pair lands in parallel
            # instead of serializing behind each other on one ring.
            nc.scalar.dma_start(out=x8a[0][:, :2, :], in_=x8d.ap()[0:P, :2, :])
            nc.scalar.dma_start(out=x8a[0][:, 2:4, :], in_=x8d.ap()[0:P, 2:4, :])
            nc.scalar.dma_start(out=x8a[0][:, 4:, :], in_=x8d.ap()[0:P, 4:, :])
            nc.scalar.dma_start(out=s8a[0][:, :4, :], in_=s8d.ap()[0:P, :4, :])
            nc.scalar.dma_start(out=s8a[0][:, 4:, :], in_=s8d.ap()[0:P, 4:, :])
            nc.sync.dma_start(out=wzr8_sb[0][:, :2, :], in_=wzr8d.ap()[0:P, :2, :])
            nc.sync.dma_start(out=wzr8_sb[0][:, 2:4, :], in_=wzr8d.ap()[0:P, 2:4, :])
            nc.sync.dma_start(out=wzr8_sb[0][:, 4:8, :], in_=wzr8d.ap()[0:P, 4:8, :])
            nc.sync.dma_start(out=wzr8_sb[0][:, 8:12, :], in_=wzr8d.ap()[0:P, 8:12, :])
            nc.sync.dma_start(out=wzr8_sb[0][:, 12:, :], in_=wzr8d.ap()[0:P, 12:, :])
            nc.sync.dma_start(out=x8a[1], in_=x8d.ap()[P : 2 * P])
            nc.sync.dma_start(out=s8a[1], in_=s8d.ap()[P : 2 * P])
            nc.sync.dma_start(out=wzr8_sb[2], in_=wzr8d.ap()[2 * P : 3 * P])
            nc.sync.dma_start(out=wh8_sb[0], in_=wh8d.ap()[0:P])
            nc.sync.dma_start(out=wx8_sb[0], in_=wx8d.ap()[0:P])
            nc.sync.dma_start(out=wx_sb[0], in_=wxt.ap()[0:P])
            nc.sync.dma_start(out=xact[0], in_=xt.ap()[0:P])
            nc.sync.dma_start(out=st_sb[0], in_=stb.ap()[:, 0, :])
            nc.sync.dma_start(out=xact[1], in_=xt.ap()[P : 2 * P])
            nc.sync.dma_start(out=wzr8_sb[1], in_=wzr8d.ap()[P : 2 * P])
            nc.sync.dma_start(out=wzr8_sb[3], in_=wzr8d.ap()[3 * P : 4 * P])
            nc.sync.dma_start(out=wh8_sb[1], in_=wh8d.ap()[P : 2 * P])
            nc.sync.dma_start(out=wx8_sb[1], in_=wx8d.ap()[P : 2 * P])
            nc.sync.dma_start(out=wx_sb[1], in_=wxt.ap()[P : 2 * P])
            nc.sync.dma_start(out=st_sb[1], in_=stb.ap()[:, 1, :])

            # Half-column sigmoid stashes, reused across the two c-rounds.
            z_st = [stash.tile([P, NF], BF16, tag=f"z{m}", name=f"z{m}") for m in range(MT)]
            r_st = [stash.tile([P, NF], BF16, tag=f"r{m}", name=f"r{m}") for m in range(MT)]

            def zr_block(g, dst):
                """One 512-col block of the fused z/r GEMM in fp8 DoubleRow:
                8 k-pair instructions contract K=2048; sigmoid (with the
                fp8 descale folded into its input scale) into dst."""
                for half in range(2):
                    accs = []
                    for mi in range(4):
                        acc = ps.tile([P, NF], F32, tag="ps", name="acc")
                        accs.append(acc)
                    for j in range(KD):
                        rhs = wzr8_sb[g][:, 2 * j : 2 * j + 2, :]
                        a = (x8a if j < 4 else s8a)[half]
                        jj = j % 4
                        for mi in range(4):
                            lhsT = a[:, 2 * jj : 2 * jj + 2, mi * P : (mi + 1) * P]
                            nc.tensor.matmul(
                                accs[mi],
                                lhsT,
                                rhs,
                                start=(j == 0),
                                stop=(j == KD - 1),
                                perf_mode=DR,
                            )
                    for mi in range(4):
                        m = half * 4 + mi
                        nc.scalar.activation(
                            dst[m],
                            accs[mi],
                            mybir.ActivationFunctionType.Sigmoid,
                            scale=1.0 / (SX * SW),
                        )

            t7 = stash.tile([P, NF], F32, tag="t7", name="t7")
            for c in range(2):  # 512-wide column block of H
                zr_block(c, z_st)       # z columns c*512..
                zr_block(2 + c, r_st)   # r columns c*512..

                if c == 1:
                    # The last tile's hh GEMM + hh*r hoist to the front of
                    # this phase: Vector is fresh here, so only the short
                    # ca/cb chains remain after the final matmul.
                    phh7 = ps.tile([P, NF], F32, tag="ps", name="phh7")
                    for k in range(KD // 2):
                        nc.tensor.matmul(
                            phh7,
                            s8a[1][:, 2 * k : 2 * k + 2, 3 * P : 4 * P],
                            wh8_sb[1][:, 2 * k : 2 * k + 2, :],
                            start=(k == 0),
                            stop=(k == KD // 2 - 1),
                            perf_mode=DR,
                        )
                    nc.vector.tensor_mul(t7, phh7, r_st[MT - 1])

                # xh & hh GEMMs + fused gate epilogue for this column block.
                # phh/pxh are scaled by SX*SW (wx pre-scaled on host), so
                # the tanh descale rides its activation input scale.
                for m in range(MT):
                    h_, r_ = divmod(m, 4)
                    msl = slice(m * P, (m + 1) * P)
                    stsl = st_sb[c][:, m * NF : (m + 1) * NF]
                    # Precompute z*state and 1-z off the critical path: the
                    # post-tanh chain is then just mul+add. These ride the
                    # otherwise-idle GpSimd / lightly-loaded Scalar engines;
                    # Vector is the h-phase bottleneck (4 ops/tile already).
                    zs = stp.tile([P, NF], BF16, tag="zs", name="zs")
                    nc.gpsimd.tensor_mul(zs, z_st[m], stsl)
                    oz = stp.tile([P, NF], BF16, tag="oz", name="oz")
                    nc.scalar.activation(
                        oz, z_st[m], mybir.ActivationFunctionType.Copy,
                        bias=1.0, scale=-1.0,
                    )

                    h = tmp.tile([P, NF], BF16, tag="h", name="h")
                    h2 = tmp.tile([P, NF], BF16, tag="h2", name="h2")
                    tb = tmp.tile([P, NF], BF16, tag="tb", name="tb")
                    if not (c == 1 and m == MT - 1):
                        phh = ps.tile([P, NF], F32, tag="ps", name="phh")
                        for k in range(KD // 2):
                            nc.tensor.matmul(
                                phh,
                                s8a[h_][:, 2 * k : 2 * k + 2, r_ * P : (r_ + 1) * P],
                                wh8_sb[c][:, 2 * k : 2 * k + 2, :],
                                start=(k == 0),
                                stop=(k == KD // 2 - 1),
                                perf_mode=DR,
                            )
                        # t = hh*r issues here so it overlaps the pxh
                        # matmuls; the post-GEMM chain is add+tanh+mul+add.
                        t = tmp.tile([P, NF], F32, tag="t", name="t")
                        nc.vector.tensor_mul(t, phh, r_st[m])
                        pxh = ps.tile([P, NF], F32, tag="ps", name="pxh")
                        nc.tensor.matmul(
                            pxh,
                            x8a[h_][:, 0:2, r_ * P : (r_ + 1) * P],
                            wx8_sb[c],
                            start=True,
                            stop=False,
                            perf_mode=DR,
                        )
                        for k in range(2, KD):
                            nc.tensor.matmul(
                                pxh,
                                xact[h_][:, k * NF + r_ * P : k * NF + (r_ + 1) * P],
                                wx_sb[c][:, k * NF : (k + 1) * NF],
                                start=False,
                                stop=(k == KD - 1),
                            )
                        # h_=tanh((xh+hh*r)/(SX*SW)); hid = h_*(1-z)+z*state
                        nc.vector.tensor_add(t, t, pxh)
                        nc.scalar.activation(
                            h, t, mybir.ActivationFunctionType.Tanh,
                            scale=1.0 / (SX * SW),
                        )
                        nc.vector.tensor_mul(h2, h, oz)
                        nc.vector.tensor_add(tb, h2, zs)
                        nc.sync.dma_start(
                            out=out.ap()[msl, c * NF : (c + 1) * NF], in_=tb
                        )
                    else:
                        # Last tile: nothing overlaps its drain, so split the
                        # xh GEMM into two accumulation groups (384+128 cols,
                        # separate PSUM tiles). The wide chunk's full chain +
                        # out-DMA runs under the narrow chunk's matmuls; only
                        # a 128-col chain + DMA remains after the last MM.
                        NA = 3 * P
                        ca, cb = slice(0, NA), slice(NA, NF)
                        pxa = ps.tile([P, NA], F32, tag="ps", name="pxa")
                        nc.tensor.matmul(
                            pxa,
                            x8a[h_][:, 0:2, r_ * P : (r_ + 1) * P],
                            wx8_sb[c][:, :, :NA],
                            start=True,
                            stop=False,
                            perf_mode=DR,
                        )
                        for k in range(2, KD):
                            nc.tensor.matmul(
                                pxa,
                                xact[h_][:, k * NF + r_ * P : k * NF + (r_ + 1) * P],
                                wx_sb[c][:, k * NF : k * NF + NA],
                                start=False,
                                stop=(k == KD - 1),
                            )
                        pxb = ps.tile([P, NF - NA], F32, tag="ps", name="pxb")
                        nc.tensor.matmul(
                            pxb,
                            x8a[h_][:, 0:2, r_ * P : (r_ + 1) * P],
                            wx8_sb[c][:, :, NA:],
                            start=True,
                            stop=False,
                            perf_mode=DR,
                        )
                        for k in range(2, KD):
                            nc.tensor.matmul(
                                pxb,
                                xact[h_][:, k * NF + r_ * P : k * NF + (r_ + 1) * P],
                                wx_sb[c][:, k * NF + NA : (k + 1) * NF],
                                start=False,
                                stop=(k == KD - 1),
                            )
                        nc.vector.tensor_add(t7[:, ca], t7[:, ca], pxa)
                        nc.scalar.activation(
                            h[:, ca], t7[:, ca],
                            mybir.ActivationFunctionType.Tanh,
                            scale=1.0 / (SX * SW),
                        )
                        nc.vector.tensor_mul(h2[:, ca], h[:, ca], oz[:, ca])
                        nc.vector.tensor_add(tb[:, ca], h2[:, ca], zs[:, ca])
                        nc.scalar.dma_start(
                            out=out.ap()[msl, c * NF : c * NF + NA],
                            in_=tb[:, ca],
                        )
                        nc.vector.tensor_add(t7[:, cb], t7[:, cb], pxb)
                        nc.scalar.activation(
                            h[:, cb], t7[:, cb],
                            mybir.ActivationFunctionType.Tanh,
                            scale=1.0 / (SX * SW),
                        )
                        nc.vector.tensor_mul(h2[:, cb], h[:, cb], oz[:, cb])
                        nc.vector.tensor_add(tb[:, cb], h2[:, cb], zs[:, cb])
                        nc.sync.dma_start(
                            out=out.ap()[msl, c * NF + NA : (c + 1) * NF],
                            in_=tb[:, cb],
                        )

    nc.compile()
    return nc


def _build_bias():
    """Bias fallback: zr GEMM in bf16 with rank-1 bias rows (the fp8
    PSUM pre-scale can't host an unscaled bias cleanly)."""
    nc = bacc.Bacc("TRN2", target_bir_lowering=False, debug=False)

    xt = nc.declare_dram_parameter("xt", [2 * P, KD * NF], BF16, isOutput=False)
    stt = nc.declare_dram_parameter("stt", [2 * P, KD * NF], BF16, isOutput=False)
    wxt = nc.declare_dram_parameter("wxt", [2 * P, KD * NF], BF16, isOutput=False)
    wht = nc.declare_dram_parameter("wht", [2 * P, KD * NF], BF16, isOutput=False)
    st = nc.declare_dram_parameter("st", [BL, H], F32, isOutput=False)
    wzrt = nc.declare_dram_parameter(
        "wzrt", [4 * P, 2 * KD * NF], BF16, isOutput=False
    )
    bzr = nc.declare_dram_parameter("bzr", [1, 2 * H], BF16, isOutput=False)
    bx = nc.declare_dram_parameter("bx", [1, H], BF16, isOutput=False)
    out = nc.declare_dram_parameter("out", [BL, H], F32, isOutput=True)

    with tile.TileContext(nc) as tc:
        with (
            tc.tile_pool(name="acts", bufs=1) as acts,
            tc.tile_pool(name="wts", bufs=1) as wts,
            tc.tile_pool(name="stash", bufs=1) as stash,
            tc.tile_pool(name="stp", bufs=8) as stp,
            tc.tile_pool(name="tmp", bufs=3) as tmp,
            tc.tile_pool(name="small", bufs=1) as small,
            tc.tile_pool(name="ps", bufs=8, space="PSUM") as ps,
        ):
            warm_sb = small.tile([P, 2 * P], BF16, tag="warm_sb")
            nc.vector.memset(warm_sb, 0.0)
            warm_ps = ps.tile([P, 2 * P], F32, tag="ps", name="warm_ps")
            for i in range(11):
                nc.tensor.matmul(
                    warm_ps, warm_sb[:, :P], warm_sb, start=True, stop=True
                )

            ones = small.tile([1, P], BF16, tag="ones")
            nc.vector.memset(ones, 1.0)
            bzr_sb = small.tile([1, 2 * H], BF16, tag="bzr")
            nc.sync.dma_start(out=bzr_sb, in_=bzr.ap())
            bx_sb = small.tile([1, H], BF16, tag="bx")
            nc.sync.dma_start(out=bx_sb, in_=bx.ap())

            xact = [acts.tile([P, KD * NF], BF16, tag=f"xa{h}", name=f"xa{h}") for h in range(2)]
            sact = [acts.tile([P, KD * NF], BF16, tag=f"sa{h}", name=f"sa{h}") for h in range(2)]
            wx_sb = [wts.tile([P, KD * NF], BF16, tag=f"wx{g}", name=f"wx{g}") for g in range(2)]
            wh_sb = [wts.tile([P, KD * NF], BF16, tag=f"wh{g}", name=f"wh{g}") for g in range(2)]
            wzr_sb = [
                wts.tile([P, 2 * KD * NF], BF16, tag=f"wzr{g}", name=f"wzr{g}")
                for g in range(4)
            ]

            def dma_cols(dst_tile, src, grp, c0, c1):
                nc.sync.dma_start(
                    out=dst_tile[:, c0:c1],
                    in_=src.ap()[grp * P : (grp + 1) * P, c0:c1],
                )

            # DMA issue order = consumption order.
            for j in (0, 1):
                dma_cols(wzr_sb[0], wzrt, 0, j * NF, (j + 1) * NF)
                dma_cols(xact[0], xt, 0, j * NF, (j + 1) * NF)
            for jc in range(1, 4):
                dma_cols(wzr_sb[0], wzrt, 0, jc * 2 * NF, (jc + 1) * 2 * NF)
                dma_cols(xact[0], xt, 0, jc * 2 * NF, (jc + 1) * 2 * NF)
            for jc in range(4):
                dma_cols(wzr_sb[0], wzrt, 0, (4 + jc) * 2 * NF, (5 + jc) * 2 * NF)
                dma_cols(sact[0], stt, 0, jc * 2 * NF, (jc + 1) * 2 * NF)
            for h in range(2):
                dma_cols(xact[1], xt, 1, h * 4 * NF, (h + 1) * 4 * NF)
            for h in range(2):
                dma_cols(sact[1], stt, 1, h * 4 * NF, (h + 1) * 4 * NF)
            dma_cols(wzr_sb[2], wzrt, 2, 0, 2 * KD * NF)
            dma_cols(wx_sb[0], wxt, 0, 0, KD * NF)
            dma_cols(wh_sb[0], wht, 0, 0, KD * NF)
            dma_cols(wzr_sb[1], wzrt, 1, 0, 2 * KD * NF)
            dma_cols(wzr_sb[3], wzrt, 3, 0, 2 * KD * NF)
            dma_cols(wx_sb[1], wxt, 1, 0, KD * NF)
            dma_cols(wh_sb[1], wht, 1, 0, KD * NF)

            def act_slice(k, m):
                t = xact if k < KD else sact
                kk = k % KD
                h, r = divmod(m, 4)
                return t[h][:, kk * NF + r * P : kk * NF + (r + 1) * P]

            z_st = [stash.tile([P, NF], BF16, tag=f"z{m}", name=f"z{m}") for m in range(MT)]
            r_st = [stash.tile([P, NF], BF16, tag=f"r{m}", name=f"r{m}") for m in range(MT)]

            def zr_block(g, dst):
                for half in range(2):
                    accs = []
                    for mi in range(4):
                        acc = ps.tile([P, NF], F32, tag="ps", name="acc")
                        accs.append(acc)
                        nc.tensor.matmul(
                            acc,
                            ones,
                            bzr_sb[:, g * NF : (g + 1) * NF],
                            start=True,
                            stop=False,
                        )
                    for k in range(2 * KD):
                        wsl = wzr_sb[g][:, k * NF : (k + 1) * NF]
                        for mi in range(4):
                            m = half * 4 + mi
                            nc.tensor.matmul(
                                accs[mi], act_slice(k, m), wsl, start=False,
                                stop=(k == 2 * KD - 1),
                            )
                    for mi in range(4):
                        m = half * 4 + mi
                        nc.scalar.activation(
                            dst[m], accs[mi], mybir.ActivationFunctionType.Sigmoid
                        )

            for c in range(2):
                csl = slice(c * NF, (c + 1) * NF)
                zr_block(c, z_st)
                zr_block(2 + c, r_st)

                for m in range(MT):
                    msl = slice(m * P, (m + 1) * P)
                    st_t = stp.tile([P, NF], F32, tag="st", name="st_t")
                    nc.sync.dma_start(out=st_t, in_=st.ap()[msl, csl])
                    zs = stp.tile([P, NF], BF16, tag="zs", name="zs")
                    nc.vector.tensor_mul(zs, z_st[m], st_t)
                    oz = stp.tile([P, NF], BF16, tag="oz", name="oz")
                    nc.scalar.activation(
                        oz, z_st[m], mybir.ActivationFunctionType.Copy,
                        bias=1.0, scale=-1.0,
                    )

                    phh = ps.tile([P, NF], F32, tag="ps", name="phh")
                    for k in range(KD):
                        nc.tensor.matmul(
                            phh,
                            act_slice(KD + k, m),
                            wh_sb[c][:, k * NF : (k + 1) * NF],
                            start=(k == 0),
                            stop=(k == KD - 1),
                        )
                    pxh = ps.tile([P, NF], F32, tag="ps", name="pxh")
                    nc.tensor.matmul(
                        pxh, ones, bx_sb[:, csl], start=True, stop=False
                    )
                    for k in range(KD):
                        nc.tensor.matmul(
                            pxh,
                            act_slice(k, m),
                            wx_sb[c][:, k * NF : (k + 1) * NF],
                            start=False,
                            stop=(k == KD - 1),
                        )

                    t = tmp.tile([P, NF], F32, tag="t", name="t")
                    h = tmp.tile([P, NF], BF16, tag="h", name="h")
                    h2 = tmp.tile([P, NF], BF16, tag="h2", name="h2")
                    nchunk = 2 if (c == 1 and m >= MT - 2) else 1
                    cw = NF // nchunk
                    for q in range(nchunk):
                        qs = slice(q * cw, (q + 1) * cw)
                        nc.vector.tensor_mul(t[:, qs], phh[:, qs], r_st[m][:, qs])
                        nc.vector.tensor_add(t[:, qs], t[:, qs], pxh[:, qs])
                        nc.scalar.activation(
                            h[:, qs], t[:, qs], mybir.ActivationFunctionType.Tanh
                        )
                        nc.vector.tensor_mul(h2[:, qs], h[:, qs], oz[:, qs])
                        nc.vector.tensor_add(t[:, qs], h2[:, qs], zs[:, qs])
                        nc.sync.dma_start(
                            out=out.ap()[msl, c * NF + q * cw : c * NF + (q + 1) * cw],
                            in_=t[:, qs],
                        )

    nc.compile()
    return nc


def _get_program(with_bias):
    key = ("nc", with_bias)
    if key not in _CACHE:
        _CACHE[key] = _build_bias() if with_bias else _build_fast()
    return _CACHE[key]


def _retile(w, ngrp):
    """[K, N] -> [ngrp*128 + p, ktile*512 + c] with w[k*128+p, g*512+c] at
    [g*128+p, k*512+c]; one partition's k-span is contiguous."""
    kt = w.shape[0] // P
    return np.ascontiguousarray(
        w.reshape(kt, P, ngrp, NF).transpose(2, 1, 0, 3).reshape(ngrp * P, kt * NF)
    )


def kernel(inp, state, wx, bx, wh, wr, ur, uz, wz, br, bz):
    import ml_dtypes

    bf16 = ml_dtypes.bfloat16
    f8 = ml_dtypes.float8_e4m3fn
    inp = np.asarray(inp, dtype=np.float32)
    state = np.asarray(state, dtype=np.float32)
    w_zr = np.block(
        [
            [np.asarray(wz, np.float32), np.asarray(wr, np.float32)],
            [np.asarray(uz, np.float32), np.asarray(ur, np.float32)],
        ]
    )
    b_zr = np.concatenate(
        [np.asarray(bz, np.float32), np.asarray(br, np.float32)]
    )[None, :]
    b_x = np.asarray(bx, np.float32)[None, :]
    with_bias = bool(np.any(b_zr) or np.any(b_x))

    if with_bias:
        common = {
            "wxt": _retile(np.asarray(wx, np.float32).astype(bf16), 2),
            "wht": _retile(np.asarray(wh, np.float32).astype(bf16), 2),
            "wzrt": _retile(w_zr.astype(bf16), 4),
            "bzr": b_zr.astype(bf16),
            "bx": np.ascontiguousarray(b_x.astype(bf16)),
        }
    else:
        common = {
            # wx pre-scaled by SX*SW so pxh matches phh's fp8 scale.
            "wxt": _retile((np.asarray(wx, np.float32) * SX * SW).astype(bf16), 2),
            "wh8": _retile((np.asarray(wh, np.float32) * SW).astype(f8), 2).reshape(
                2 * P, KD, NF
            ),
            "wx8": _retile(
                (np.asarray(wx, np.float32)[: 2 * P] * SW).astype(f8), 2
            ).reshape(2 * P, 2, NF),
            "wzr8": _retile((w_zr * SW).astype(f8), 4).reshape(4 * P, 2 * KD, NF),
        }

    in_maps = []
    for cidx in range(N_CORES):
        sl = slice(cidx * BL, (cidx + 1) * BL)
        xT = np.ascontiguousarray(inp[sl].T)  # [D, BL]
        im = {"xt": _retile(xT.astype(bf16), 2), **common}
        if with_bias:
            sT = np.ascontiguousarray(state[sl].T)
            im["stt"] = _retile(sT.astype(bf16), 2)
            im["st"] = np.ascontiguousarray(state[sl])
        else:
            sT = np.ascontiguousarray(state[sl].T)
            im["x8"] = _retile((xT * SX).astype(f8), 2).reshape(2 * P, KD, NF)
            im["s8"] = _retile((sT * SX).astype(f8), 2).reshape(2 * P, KD, NF)
            # state for the gate epilogue: [p, c, m*512+col] bf16
            im["stb"] = np.ascontiguousarray(
                state[sl]
                .astype(bf16)
                .reshape(MT, P, 2, NF)
                .transpose(1, 2, 0, 3)
                .reshape(P, 2, MT * NF)
            )
        in_maps.append(im)

    nc = _get_program(with_bias)
    trace = bool(int(os.environ.get("GRU_TRACE", "0")))
    res = run_bass_kernel_spmd(nc, in_maps, list(range(N_CORES)), trace=trace)
    if trace:
        _CACHE["last_exec_time_ns"] = res.exec_time_ns
        _CACHE["last_results"] = res
    return np.concatenate(
        [np.asarray(res.results[c]["out"], dtype=np.float32) for c in range(N_CORES)],
        axis=0,
    )
